# revision 34
# baseline (speedup 1.0000x reference)
"""Causal self-attention (RoPE, 16 heads) Trainium2 Bass kernel — v3.

Problem: B=8, S=1024, D=1024, H=16, HS=64, fp32 in/out, causal mask.
Data-parallel over batch — one batch element per NeuronCore; bf16 matmuls.

v3 redesign vs v2 (365µs baseline):
  - Score matmuls run 2 heads CONCURRENTLY via tile_position row-tiling
    (K=64 each: head A rows 0-63, head B rows 64-127) -> ~2x score phase.
  - Score PSUM tiles are [128, 1024] (2 banks, full q span per key block)
    so each exp is ONE wide ACTIVATE -> halves ACT instruction overhead.
  - Causal diag masking via DVE tensor_mask (per-partition threshold) on
    strided multi-block views -> ~6 instrs/ft instead of 16 tensor muls.
  - RoPE reads the QK projection PSUM directly (no praw copy); the c2-mul
    and final add run on GPSIMD; only shuffle + c1-mul stay on DVE.
  - Software pipelining: QKproj(ft+1) is emitted inside ft's body so the
    PE never idles waiting for RoPE (v2 lost ~63µs idle + ~53µs HAM
    re-throttle penalty to this).
  - V-phase PSUM->SBUF copies moved to the (then idle) scalar engine.
  - V phase ping-pongs through halves of one [128,1024] PSUM tile so the
    whole kernel fits exactly in 8 PSUM banks:
      pssA(2) + pssB(2) + qkps(2) + psyA(1) + psyB(1) = 8.
"""

import os

if "axon" not in os.environ.get("JAX_PLATFORMS", "axon"):
    os.environ.pop("JAX_PLATFORMS", None)

import numpy as np
import ml_dtypes
from contextlib import ExitStack

import concourse.bass as bass
import concourse.mybir as mybir
import concourse.tile as tile
from concourse import bacc
from concourse.bass_utils import run_bass_kernel_spmd

B, S, D, H, HS = 8, 1024, 1024, 16, 64
P = 128
NCORES = 8
F32 = mybir.dt.float32
BF = mybir.dt.bfloat16
EXP = mybir.ActivationFunctionType.Exp
NPBF = ml_dtypes.bfloat16

# out lane i <- in lane (i+16) % 32, same permutation in every 32-group
SHUF = [(i + 16) % 32 for i in range(32)]

# Fallback switches (default = full v3 path)
GP_ROPE = os.environ.get("GP_ROPE", "1") == "1"      # c2-mul + add on GPSIMD
TMASK = os.environ.get("TMASK", "1") == "1"          # tensor_mask for causal diag
# stream_shuffle cannot read PSUM (walrus ISA check) — praw copy path default
PSUM_SHUF = os.environ.get("PSUM_SHUF", "0") == "1"
TPOS = os.environ.get("TPOS", "1") == "1"            # row-tiled score matmuls
PSUM_RECIP = os.environ.get("PSUM_RECIP", "1") == "1"  # recip direct from PSUM p0
MASK_GP = os.environ.get("MASK_GP", "1") == "1"      # masks on GPSIMD
DEBUG_DUMP = os.environ.get("DEBUG_DUMP", "0") == "1"

_CACHE = {}


def _build_nc():
    nc = bacc.Bacc(
        "TRN2", target_bir_lowering=False, debug=False, num_devices=NCORES)
    x_d = nc.dram_tensor("x", [D, S], BF, kind="ExternalInput")  # x^T
    wq_d = nc.dram_tensor("wq", [D, D], BF, kind="ExternalInput")
    wk_d = nc.dram_tensor("wk", [D, D], BF, kind="ExternalInput")
    wv_d = nc.dram_tensor("wv", [D, D], BF, kind="ExternalInput")
    wp_d = nc.dram_tensor("wp", [D, D], BF, kind="ExternalInput")
    c1q_d = nc.dram_tensor("c1q", [P, S], BF, kind="ExternalInput")
    c2q_d = nc.dram_tensor("c2q", [P, S], BF, kind="ExternalInput")
    maskr_d = nc.dram_tensor("maskr", [P, 3 * P], BF, kind="ExternalInput")
    maskt_d = nc.dram_tensor("maskt", [P, P], BF, kind="ExternalInput")
    out_d = nc.dram_tensor("out", [S, D], F32, kind="ExternalOutput")
    if DEBUG_DUMP:
        dbg = {
            "dvt": nc.dram_tensor("dvt", [P, H * 2 * HS], BF, kind="ExternalOutput"),
            "dqt": nc.dram_tensor("dqt", [P, S], BF, kind="ExternalOutput"),
            "dkt": nc.dram_tensor("dkt", [P, S], BF, kind="ExternalOutput"),
            "datt": nc.dram_tensor("datt", [P, 4096], BF, kind="ExternalOutput"),
            "drb": nc.dram_tensor("drb", [P, 512], F32, kind="ExternalOutput"),
            "dyt": nc.dram_tensor("dyt", [P, S], BF, kind="ExternalOutput"),
        }

    def mm(out, lhsT, rhs, start, stop, tp=None):
        nc.tensor.matmul(out, lhsT, rhs, start=start, stop=stop,
                         tile_position=tp)

    with tile.TileContext(nc) as tc, ExitStack() as ctx:
        persist = ctx.enter_context(tc.tile_pool(name="persist", bufs=1))
        # vt cols 0-63 = 64 replicated ones columns (the AV matmul then
        # emits 64 identical softmax-denominator rows into PSUM rows 0-63 —
        # matmul-side broadcast), cols 64-127 = the v head values. Denominator
        # first: custom DVE ops (reciprocal) only work at partition base 0.
        vt = [persist.tile([P, H, 2 * HS], BF, name=f"vt{i}", tag=f"vt{i}")
              for i in range(8)]
        yt = [persist.tile([P, S], BF, name=f"yt{i}", tag=f"yt{i}") for i in range(8)]
        c1q = persist.tile([P, S], BF, name="c1q_t", tag="c1q_t")
        c2q = persist.tile([P, S], BF, name="c2q_t", tag="c2q_t")
        maskr = persist.tile([P, 3 * P], BF, name="maskr", tag="maskr")
        maskt = persist.tile([P, P], BF, name="maskt", tag="maskt")
        for t_, d_ in ((c1q, c1q_d), (c2q, c2q_d),
                       (maskr, maskr_d), (maskt, maskt_d)):
            nc.sync.dma_start(t_[:], d_[:])
        wpt = []
        for dc in range(8):
            wtile = persist.tile([P, D], BF, name=f"wpt{dc}", tag=f"wpt{dc}")
            wpt.append(wtile)

        with ExitStack() as pctx:
            xtp = pctx.enter_context(tc.tile_pool(name="xtp", bufs=1))
            wqkp = pctx.enter_context(tc.tile_pool(name="wqkp", bufs=1))
            qkt_p = pctx.enter_context(tc.tile_pool(name="qkt", bufs=2))
            attp = pctx.enter_context(tc.tile_pool(name="attp", bufs=2))
            ropep = pctx.enter_context(tc.tile_pool(name="ropep", bufs=2))
            smallp = pctx.enter_context(tc.tile_pool(name="smallp", bufs=2))
            pssA_p = pctx.enter_context(
                tc.tile_pool(name="pssA", bufs=1, space="PSUM"))
            pssB_p = pctx.enter_context(
                tc.tile_pool(name="pssB", bufs=1, space="PSUM"))
            qk_p = pctx.enter_context(
                tc.tile_pool(name="qkp", bufs=1, space="PSUM"))
            psyA_p = pctx.enter_context(
                tc.tile_pool(name="psyA", bufs=1, space="PSUM"))
            psyB_p = pctx.enter_context(
                tc.tile_pool(name="psyB", bufs=1, space="PSUM"))

            # DMA order = consumption order: x+wv (V phase), wq, wk, wp.
            xt, wvt, wqt, wkt = [], [], [], []
            with ExitStack() as vctx:
                wvp = vctx.enter_context(tc.tile_pool(name="wvp", bufs=1))
                for dc in range(8):
                    xtile = xtp.tile([P, S], BF, name=f"xt{dc}", tag=f"xt{dc}")
                    nc.sync.dma_start(xtile[:], x_d[dc * P:(dc + 1) * P, :])
                    xt.append(xtile)
                    wtile = wvp.tile([P, D], BF, name=f"wvt{dc}", tag=f"wvt{dc}")
                    nc.sync.dma_start(wtile[:], wv_d[dc * P:(dc + 1) * P, :])
                    wvt.append(wtile)
                for dc in range(8):
                    wtile = wqkp.tile([P, D], BF, name=f"wqt{dc}", tag=f"wqt{dc}")
                    nc.sync.dma_start(wtile[:], wq_d[dc * P:(dc + 1) * P, :])
                    wqt.append(wtile)
                for dc in range(8):
                    wtile = wqkp.tile([P, D], BF, name=f"wkt{dc}", tag=f"wkt{dc}")
                    nc.sync.dma_start(wtile[:], wk_d[dc * P:(dc + 1) * P, :])
                    wkt.append(wtile)
                for dc in range(8):
                    nc.sync.dma_start(wpt[dc][:], wp_d[dc * P:(dc + 1) * P, :])

                # ---------- rope helper: dst = ps*c1 + shuffle(ps)*c2 ----------
                # gp_tail: how many trailing elementwise ops to push to GPSIMD
                # (its TT is ~3x slower than DVE, so only chain tails go there)
                def emit_rope(ps, dst, c1, c2, gp_tail=1):
                    t = ropep.tile([P, S], BF, name="ropet", tag="rt")
                    if PSUM_SHUF:
                        nc.vector.stream_shuffle(t[:], ps[:], SHUF)
                        nc.vector.tensor_mul(dst[:], ps[:], c1[:])
                    else:
                        praw = ropep.tile([P, S], BF, name="praw", tag="praw", bufs=1)
                        nc.vector.tensor_copy(praw[:], ps[:])
                        nc.vector.stream_shuffle(t[:], praw[:], SHUF)
                        nc.vector.tensor_mul(dst[:], praw[:], c1[:])
                    t2 = ropep.tile([P, S], BF, name="ropet2", tag="rt2", bufs=1)
                    if not GP_ROPE:
                        gp_tail = 0
                    eng_mul = nc.gpsimd if gp_tail >= 2 else nc.vector
                    eng_add = nc.gpsimd if gp_tail >= 1 else nc.vector
                    eng_mul.tensor_mul(t2[:], t[:], c2[:])
                    eng_add.tensor_add(dst[:], dst[:], t2[:])

                def emit_qkproj_mms(ps, wt, ft):
                    for s2 in range(2):
                        for dc in range(8):
                            mm(ps[:, s2 * 512:(s2 + 1) * 512],
                               wt[dc][:, ft * P:(ft + 1) * P],
                               xt[dc][:, s2 * 512:(s2 + 1) * 512],
                               dc == 0, dc == 7)

                # ---------- Phase 0: V = x @ wv (+ QKproj(0) interleaved) ------
                for sc in range(8):
                    nc.vector.memset(vt[sc][:, :, 0:HS], 1.0)
                vps = qk_p.tile([P, S], F32, name="vps", tag="qk")
                qt0 = kt0 = ps_q0 = ps_k0 = None
                for sc in range(8):
                    for f2 in range(2):
                        psl = vps[:, f2 * 512:(f2 + 1) * 512]
                        for dc in range(8):
                            mm(psl, xt[dc][:, sc * P:(sc + 1) * P],
                               wvt[dc][:, f2 * 512:(f2 + 1) * 512], dc == 0, dc == 7)
                        nc.scalar.copy(
                            vt[sc][:, f2 * 8:(f2 + 1) * 8, HS:2 * HS],
                            psl.rearrange("p (h e) -> p h e", e=HS))
                    if sc == 3:
                        # Q projection of ft0 into the pssA-pool bank pair
                        ps_q0 = pssA_p.tile([P, S], F32, name="pssA", tag="pssA")
                        emit_qkproj_mms(ps_q0, wqt, 0)
                    if sc == 5:
                        ps_k0 = pssB_p.tile([P, S], F32, name="pssB", tag="pssB")
                        emit_qkproj_mms(ps_k0, wkt, 0)
                qt0 = qkt_p.tile([P, S], BF, name="qt", tag="qt")
                kt0 = qkt_p.tile([P, S], BF, name="kt", tag="kt")
                emit_rope(ps_q0, qt0, c1q, c2q, gp_tail=1)
                emit_rope(ps_k0, kt0, c1q, c2q, gp_tail=2)
                if DEBUG_DUMP:
                    nc.sync.dma_start(
                        dbg["dvt"][:], vt[0][:].rearrange("p h e -> p (h e)"))
                    nc.sync.dma_start(dbg["dqt"][:], qt0[:])
                    nc.sync.dma_start(dbg["dkt"][:], kt0[:])

            # ---------------- Phase 1: per head-pair ft ----------------
            qt_cur, kt_cur = qt0, kt0

            def emit_exp(pss, att_g0, att_g1, h, kc):
                n0e = kc * P
                if kc < 4:
                    dst = att_g0[:, h, kc * 1024 + n0e:(kc + 1) * 1024]
                else:
                    dst = att_g1[:, h, (kc - 4) * 512 + (n0e - 512):(kc - 3) * 512]
                nc.scalar.activation(dst, pss[:, n0e:], EXP,
                                     scale=float(1.0 / np.sqrt(HS)))

            def _emit_masks(att_g, h, stride):
                # diag block kc sits at flat offset kc*stride; zero its upper
                # triangle. One strided 3-block mul + one single-block mul.
                eng = nc.gpsimd if MASK_GP else nc.vector
                if TMASK:
                    v = att_g[:, h, 0:3 * stride].rearrange(
                        "p (b c) -> p b c", c=stride)[:, :, 0:P]
                    m3 = maskr[:, 0:3 * P].rearrange("p (b c) -> p b c", c=P)
                    eng.tensor_mul(v, v, m3)
                    v1 = att_g[:, h, 3 * stride:3 * stride + P]
                    eng.tensor_mul(v1, v1, maskt[:])
                else:
                    for kc in range(4):
                        sl = att_g[:, h, kc * stride:kc * stride + P]
                        eng.tensor_mul(sl, sl, maskt[:])

            def emit_masks_g0(att_g0, h):
                _emit_masks(att_g0, h, 1152)

            def emit_masks_g1(att_g1, h):
                _emit_masks(att_g1, h, 640)

            def att_slice(att_g0, att_g1, h, kc, qc):
                # columns [qc*512 + n0, (qc+1)*512) of key-block kc's att row
                n0 = max(kc * P - qc * 512, 0)
                if kc < 4:
                    return att_g0[:, h, kc * 1024 + qc * 512 + n0:
                                  kc * 1024 + (qc + 1) * 512]
                return att_g1[:, h, (kc - 4) * 512 + n0:(kc - 3) * 512]

            _dumped = []

            def emit_norm(psy, ft, hb, qc):
                # psy rows 0-63 = 64 copies of the softmax denominator
                # (replicated ones columns of V); rows 64-127 = unnorm. y.
                rb = smallp.tile([64, 512], F32, name="rb", tag="rb")
                if PSUM_RECIP:
                    nc.vector.reciprocal_approx_fast(
                        out=rb[:], in_=psy[0:64, :])
                else:
                    den = smallp.tile([64, 512], F32, name="den", tag="den",
                                      bufs=1)
                    nc.vector.tensor_copy(den[:], psy[0:64, :])
                    nc.vector.reciprocal_approx_fast(out=rb[:], in_=den[:])
                if DEBUG_DUMP and ft == 0 and hb == 0 and qc == 0:
                    _dumped.append(1)
                    nc.sync.dma_start(dbg["drb"][0:64, :], rb[:])
                nc.vector.tensor_mul(
                    yt[ft][hb:hb + 64, qc * 512:(qc + 1) * 512],
                    psy[64:128, :], rb[:])

            for ft in range(8):
                att_g0 = attp.tile([P, 2, 4096], BF, name="att0", tag="att0")
                att_g1 = attp.tile([P, 2, 2048], BF, name="att1", tag="att1")

                def emit_scores(kc):
                    pssA = pssA_p.tile([P, S], F32, name="pssA", tag="pssA")
                    pssB = pssB_p.tile([P, S], F32, name="pssB", tag="pssB")
                    for qh in range(2):
                        n0 = min(max(kc * P - qh * 512, 0), 512)
                        if n0 >= 512:
                            continue
                        sl = slice(qh * 512 + n0, (qh + 1) * 512)
                        tpA = (0, 0) if TPOS else None
                        tpB = (64, 0) if TPOS else None
                        mm(pssA[:, sl], kt_cur[0:64, kc * P:(kc + 1) * P],
                           qt_cur[0:64, sl], True, True, tp=tpA)
                        mm(pssB[:, sl], kt_cur[64:128, kc * P:(kc + 1) * P],
                           qt_cur[64:128, sl], True, True, tp=tpB)
                    emit_exp(pssA, att_g0, att_g1, 0, kc)
                    emit_exp(pssB, att_g0, att_g1, 1, kc)

                # kc 0-1 + K-projection of ft+1
                emit_scores(0)
                emit_scores(1)
                ps_k = ps_q = None
                if ft < 7:
                    ps_k = qk_p.tile([P, S], F32, name="kps", tag="qk")
                    emit_qkproj_mms(ps_k, wkt, ft + 1)
                    kt_nxt = qkt_p.tile([P, S], BF, name="kt", tag="kt")
                    emit_rope(ps_k, kt_nxt, c1q, c2q, gp_tail=2)
                emit_scores(2)
                emit_scores(3)
                emit_masks_g0(att_g0, 0)
                emit_masks_g0(att_g0, 1)

                if ft < 7:
                    ps_q = qk_p.tile([P, S], F32, name="qps", tag="qk")
                    emit_qkproj_mms(ps_q, wqt, ft + 1)
                    qt_nxt = qkt_p.tile([P, S], BF, name="qt", tag="qt")
                    emit_rope(ps_q, qt_nxt, c1q, c2q, gp_tail=1)

                # AV qc0 (key blocks 0-3)
                psyA = psyA_p.tile([P, 512], F32, name="psyA", tag="psyA")
                psyB = psyB_p.tile([P, 512], F32, name="psyB", tag="psyB")
                for kc in range(4):
                    n0 = kc * P
                    mm(psyA[:, n0:], vt[kc][:, 2 * ft, :],
                       att_slice(att_g0, att_g1, 0, kc, 0), kc == 0, kc == 3)
                    mm(psyB[:, n0:], vt[kc][:, 2 * ft + 1, :],
                       att_slice(att_g0, att_g1, 1, kc, 0), kc == 0, kc == 3)

                emit_scores(4)
                emit_scores(5)
                emit_norm(psyA, ft, 0, 0)
                emit_norm(psyB, ft, 64, 0)
                emit_scores(6)
                emit_scores(7)

                # AV qc1: key blocks 0-3 first (their diagonals live in qc0's
                # columns, so no mask dependency), then the g1 masks, then 4-7.
                psyA = psyA_p.tile([P, 512], F32, name="psyA", tag="psyA")
                psyB = psyB_p.tile([P, 512], F32, name="psyB", tag="psyB")
                for kc in range(4):
                    mm(psyA[:], vt[kc][:, 2 * ft, :],
                       att_slice(att_g0, att_g1, 0, kc, 1), kc == 0, False)
                    mm(psyB[:], vt[kc][:, 2 * ft + 1, :],
                       att_slice(att_g0, att_g1, 1, kc, 1), kc == 0, False)
                emit_masks_g1(att_g1, 0)
                emit_masks_g1(att_g1, 1)
                for kc in range(4, 8):
                    n0 = kc * P - 512
                    mm(psyA[:, n0:], vt[kc][:, 2 * ft, :],
                       att_slice(att_g0, att_g1, 0, kc, 1), False, kc == 7)
                    mm(psyB[:, n0:], vt[kc][:, 2 * ft + 1, :],
                       att_slice(att_g0, att_g1, 1, kc, 1), False, kc == 7)
                emit_norm(psyA, ft, 0, 1)
                emit_norm(psyB, ft, 64, 1)
                if DEBUG_DUMP and ft == 0:
                    nc.sync.dma_start(dbg["datt"][:], att_g0[:, 0, :])
                    nc.sync.dma_start(dbg["dyt"][:], yt[0][:])
                if ft < 7:
                    kt_cur, qt_cur = kt_nxt, qt_nxt

            # ---------- Phase 2: output projection (same pool ctx — no
            # pool-close barrier between attention tail and projection) ----
            proj_psA = pssA_p.tile([P, S], F32, name="pssA", tag="pssA")
            proj_psB = pssB_p.tile([P, S], F32, name="pssB", tag="pssB")
            quarters = [proj_psA[:, 0:512], proj_psA[:, 512:1024],
                        proj_psB[:, 0:512], proj_psB[:, 512:1024]]
            i = 0
            for n2 in range(2):
                for sc in range(8):
                    psp = quarters[i % 4]
                    i += 1
                    for dc in range(8):
                        mm(psp, yt[dc][:, sc * P:(sc + 1) * P],
                           wpt[dc][:, n2 * 512:(n2 + 1) * 512], dc == 0, dc == 7)
                    ot = smallp.tile([P, 512], F32, name="ot", tag="ot", bufs=1)
                    nc.scalar.copy(ot[:], psp)
                    nc.sync.dma_start(
                        out_d[sc * P:(sc + 1) * P, n2 * 512:(n2 + 1) * 512], ot[:])
    nc.compile()
    return nc


def _prep(inputs):
    w_qkv = np.asarray(inputs["w_qkv"], np.float32)
    w_proj = np.asarray(inputs["w_proj"], np.float32)
    cos = np.asarray(inputs["cos"], np.float32).reshape(S, HS // 2)
    sin = np.asarray(inputs["sin"], np.float32).reshape(S, HS // 2)
    wq, wk, wv = w_qkv[:, 0:D], w_qkv[:, D:2 * D], w_qkv[:, 2 * D:3 * D]

    cosT = np.ascontiguousarray(cos.T)  # [32, S] freq-major
    sinT = np.ascontiguousarray(sin.T)
    perm = np.empty(D, np.int64)
    c1 = np.empty((P, S), np.float32)
    c2 = np.empty((P, S), np.float32)
    # Deinterleave rope pairs so x1/x2 of pair j sit 16 lanes apart
    # inside a 32-lane group: rows [32g:32g+16] = x1 of pairs
    # 16(g%2)+0..15 (features 2j), rows [32g+16:32g+32] = x2 (2j+1).
    for h in range(H):
        b0 = h * HS
        for g in range(2):
            base = b0 + 32 * g
            js = 16 * g + np.arange(16)
            perm[base:base + 16] = b0 + 2 * js
            perm[base + 16:base + 32] = b0 + 2 * js + 1
    for g in range(4):
        hh = g % 2
        c1[32 * g:32 * g + 16] = cosT[16 * hh:16 * hh + 16]
        c1[32 * g + 16:32 * g + 32] = cosT[16 * hh:16 * hh + 16]
        c2[32 * g:32 * g + 16] = -sinT[16 * hh:16 * hh + 16]
        c2[32 * g + 16:32 * g + 32] = sinT[16 * hh:16 * hh + 16]
    wq, wk = wq[:, perm], wk[:, perm]
    mask = np.triu(np.ones((P, P), np.float32))  # [k, q]: allow q >= k
    common = {
        "wq": np.ascontiguousarray(wq).astype(NPBF),
        "wk": np.ascontiguousarray(wk).astype(NPBF),
        "wv": np.ascontiguousarray(wv).astype(NPBF),
        "wp": np.ascontiguousarray(w_proj).astype(NPBF),
        "c1q": c1.astype(NPBF), "c2q": c2.astype(NPBF),
        "maskr": np.tile(mask, (1, 3)).astype(NPBF),
        "maskt": mask.astype(NPBF),
    }
    return common


LAST_RESULT = None


def kernel(**inputs):
    global LAST_RESULT
    if "nc" not in _CACHE:
        _CACHE["nc"] = _build_nc()
    nc = _CACHE["nc"]
    common = _prep(inputs)
    x = np.asarray(inputs["x"], np.float32)
    in_maps = [dict(common, x=x[b].T.astype(NPBF)) for b in range(B)]
    res = run_bass_kernel_spmd(nc, in_maps, list(range(NCORES)))
    LAST_RESULT = res
    out = np.stack([res.results[i]["out"] for i in range(B)], 0)
    return out.astype(np.float32)
